# revision 5
# baseline (speedup 1.0000x reference)
"""Trainium2 Bass kernel for multi-head attention (B=4, N=2048, C=512, 8 heads).

Sharding: 8 cores = (batch b = core//2) x (head-group g = core%2, 4 heads each).
Per core, a transposed-scores attention pipeline:
  - host supplies x[b] transposed (xT [C, N]) and per-group transposed weights,
    all pre-cast to fp16 (matmul streams at 1 cycle/row; ~4x the mantissa of
    bf16; every tensor here fits fp16 range comfortably)
  - qT/kT stored zero-padded per head ([:, hh, :] has head hh's 64 dims on
    its own partition range, rest zero) so score matmuls contract over the
    full K=128 partition range: same N cycles as K=64, but the PE activity
    monitor sees a fully-active array and keeps the 2.4 GHz clock (K=64
    matmuls measure at the 1.2 GHz throttled rate)
  - v as [N, (1+64) per head] tiles; the leading ones column makes attn@v
    emit the softmax denominator into PSUM partition 0
  - the ACT exp stream is the pacing engine (128 x [128,1024] exps ~ 142us);
    everything else is scheduled around keeping it gap-free:
      * DMA order puts wk + xT first-halves first so the QK projections
        (and hence the first exp) start as early as possible
      * kT/qT zero-pads go to DVE (pair 0) and GpSimd (pair 1) up front,
        off the DMA/PE critical path
      * sections run qh-major so the output projection (which needs all
        heads for a token range) can interleave with the qh=1 sections
      * phase-A leftovers + y-projection blocks trickle in as per-block
        fillers with an explicit deps-aware plan (~1 filler/block, placed
        after the block's scores so they never delay the next exp)
  - normalization entirely off the PE: DVE fast-reciprocal (partition 0),
    GpSimd partition_broadcast, DVE multiply, DMA partition-shift into outT
  - host sums the two half-head partials
"""

import sys

sys.path.insert(0, "/opt/trn_rl_repo")

import numpy as np

B, N, C = 4, 2048, 512
H, D = 8, 64
SCALE = float(D) ** -0.5  # 0.125, exact in fp32
P = 128
CT = C // P  # 4 contraction tiles over channels
NT = N // P  # 16 token blocks
NCORES = 8
FD = 1024  # softmax block free dim (q chunk)
QH = N // FD  # 2 q halves

_cache = {}


def _build():
    import concourse.bacc as bacc
    import concourse.tile as tile
    from concourse import mybir

    f32 = mybir.dt.float32
    f16 = mybir.dt.float16
    u16 = mybir.dt.uint16
    EXP = mybir.ActivationFunctionType.Exp

    nc = bacc.Bacc("TRN2", target_bir_lowering=False, debug=False,
                   num_devices=NCORES)

    xT_d = nc.dram_tensor("xT", [C, N], f16, kind="ExternalInput")
    wqT_d = nc.dram_tensor("wqT", [P, CT * 256], f16, kind="ExternalInput")
    wkT_d = nc.dram_tensor("wkT", [P, CT * 256], f16, kind="ExternalInput")
    wvT_d = nc.dram_tensor("wvT", [P, CT * 256], f16, kind="ExternalInput")
    pwT_d = nc.dram_tensor("pwT", [P, 2 * C], f16, kind="ExternalInput")
    y_d = nc.dram_tensor("y", [N, C], f32, kind="ExternalOutput")

    with tile.TileContext(nc) as tc:
        with (
            tc.tile_pool(name="io", bufs=1) as io,
            tc.tile_pool(name="qk", bufs=1) as qk,
            tc.tile_pool(name="expp", bufs=6) as expp,
            tc.tile_pool(name="workp", bufs=3) as workp,
            tc.tile_pool(name="yp", bufs=4) as yp,
            tc.tile_pool(name="ps_s", bufs=2, space="PSUM") as ps_s,
            tc.tile_pool(name="ps_o", bufs=2, space="PSUM") as ps_o,
        ):
            # ---- input loads: wk first (gates the first projection), then
            # the xT first halves (token 0:1024 per channel tile), then wq,
            # wv, the xT second halves, pw ----
            xT_sb = io.tile([P, CT, N], f16, tag="xT", name="xT_sb")
            xT_ap = xT_d[:].rearrange("(t p) n -> p t n", p=P)

            wk_sb = io.tile([P, CT, 256], f16, tag="wk", name="wk_sb")
            nc.sync.dma_start(
                wk_sb[:], wkT_d[:].rearrange("p (t m) -> p t m", t=CT))
            for t in range(CT):
                nc.sync.dma_start(xT_sb[:, t, 0:1024], xT_ap[:, t, 0:1024])
            wq_sb = io.tile([P, CT, 256], f16, tag="wq", name="wq_sb")
            nc.sync.dma_start(
                wq_sb[:], wqT_d[:].rearrange("p (t m) -> p t m", t=CT))
            wv_sb = io.tile([P, CT, 256], f16, tag="wv", name="wv_sb")
            nc.sync.dma_start(
                wv_sb[:], wvT_d[:].rearrange("p (t m) -> p t m", t=CT))
            for t in range(CT):
                nc.sync.dma_start(xT_sb[:, t, 1024:2048],
                                  xT_ap[:, t, 1024:2048])
            pw_sb = io.tile([P, 2, C], f16, tag="pw", name="pw_sb")
            nc.sync.dma_start(
                pw_sb[:], pwT_d[:].rearrange("p (t m) -> p t m", t=2))

            # ---- SBUF persistents ----
            qT = []
            kT = []
            vv = []
            outT = []
            for p in range(2):
                qT.append(qk.tile([P, 2, N], f16, tag=f"qT{p}", name=f"qT{p}"))
                kT.append(qk.tile([P, 2, N], f16, tag=f"kT{p}", name=f"kT{p}"))
                vv.append(qk.tile([P, NT, 130], f16, tag=f"v{p}", name=f"v{p}"))
                outT.append(qk.tile([P, N], f16, tag=f"outT{p}", name=f"outT{p}"))

            # trigger the ACT exp table load during the DMA ramp
            scratch1 = io.tile([1, 2], f32, tag="scratch1", name="scratch1")
            nc.vector.memset(scratch1[:], 0.0)
            nc.scalar.activation(scratch1[0:1, 0:1], scratch1[0:1, 1:2], EXP)
            # zero-pad halves: pair 0 on DVE (fast, needed by the first
            # scores ~15us in), pair 1 on GpSimd (slow but idle until the
            # first norm; needed only by section 2 ~60us in)
            nc.vector.memset(kT[0][64:128, 0, :], 0.0)
            nc.vector.memset(kT[0][0:64, 1, :], 0.0)
            nc.vector.memset(qT[0][64:128, 0, :], 0.0)
            nc.vector.memset(qT[0][0:64, 1, :], 0.0)
            nc.gpsimd.memset(kT[1][64:128, 0, :], 0.0)
            nc.gpsimd.memset(kT[1][0:64, 1, :], 0.0)
            nc.gpsimd.memset(qT[1][64:128, 0, :], 0.0)
            nc.gpsimd.memset(qT[1][0:64, 1, :], 0.0)
            for p in range(2):
                # ones columns (fp16 1.0) at the head of each v block (DVE:
                # strided 16-element memsets are fast there, slow on GpSimd)
                nc.vector.memset(vv[p][:, :, 0:1].bitcast(u16), 0x3C00)
                nc.vector.memset(vv[p][:, :, 65:66].bitcast(u16), 0x3C00)

            def emit_qk_chunk(p, w_sb, dst, ch, dve_only=False):
                pc = slice(128 * p, 128 * (p + 1))
                cs = slice(512 * ch, 512 * (ch + 1))
                ps = ps_s.tile([P, FD], f32, tag="s",
                               name=f"qkps_{p}_{ch}_{w_sb.tensor.name}")
                for t in range(CT):
                    nc.tensor.matmul(
                        ps[:, :512],
                        lhsT=w_sb[:, t, pc],
                        rhs=xT_sb[:, t, cs],
                        start=(t == 0), stop=(t == CT - 1))
                nc.vector.tensor_copy(dst[0:64, 0, cs], ps[0:64, :512])
                if dve_only:
                    nc.vector.tensor_copy(dst[64:128, 1, cs], ps[64:128, :512])
                else:
                    nc.scalar.copy(dst[64:128, 1, cs], ps[64:128, :512])

            def emit_v_tile(tt):
                psv = ps_s.tile([P, FD], f32, tag="s", name=f"vps_{tt}")
                for t in range(CT):
                    nc.tensor.matmul(
                        psv[:, :256],
                        lhsT=xT_sb[:, t, 128 * tt:128 * (tt + 1)],
                        rhs=wv_sb[:, t, 0:256],
                        start=(t == 0), stop=(t == CT - 1))
                for p in range(2):
                    pv = psv[:, 128 * p:128 * (p + 1)].rearrange(
                        "p (two d) -> p two d", two=2)
                    dv = vv[p][:, tt, 0:130].rearrange(
                        "p (two d65) -> p two d65", two=2)[:, :, 1:65]
                    nc.vector.tensor_copy(dv, pv)

            def emit_y_block(tt, act_evict=False):
                yps = ps_s.tile([P, FD], f32, tag="s", name=f"yps_{tt}")
                for p in range(2):
                    nc.tensor.matmul(
                        yps[:, :512], lhsT=outT[p][:, 128 * tt:128 * (tt + 1)],
                        rhs=pw_sb[:, p, :], start=(p == 0), stop=(p == 1))
                ys = yp.tile([P, C], f32, tag="y", name=f"ys_{tt}")
                if act_evict:
                    nc.scalar.copy(ys[:], yps[:, :512])
                else:
                    nc.vector.tensor_copy(ys[:], yps[:, :512])
                nc.sync.dma_start(y_d[128 * tt:128 * (tt + 1), :], ys[:])

            def norm_head(p, hh, qh, o):
                qs = slice(FD * qh, FD * (qh + 1))
                r = workp.tile([P, FD], f32, tag="r", name=f"r_{p}_{hh}_{qh}")
                nc.vector.reciprocal_approx_fast(r[0:1, :], o[0:1, :])
                rb = workp.tile([65, FD], f32, tag="rb",
                                name=f"rb_{p}_{hh}_{qh}")
                nc.gpsimd.partition_broadcast(rb[:], r[0:1, :])
                st = workp.tile([65, FD], f16, tag="st",
                                name=f"st_{p}_{hh}_{qh}")
                nc.vector.tensor_mul(st[:], o[:], rb[:])
                nc.sync.dma_start(outT[p][64 * hh:64 * (hh + 1), qs],
                                  st[1:65, :])

            def emit_section(p, hh, qh, fillers):
                """fillers: list of per-block lists of thunks (len NT+1);
                fillers[i] runs after block i's scores+exp, before
                attnv(i-1); fillers[NT] runs before the trailing attnv."""
                vs = slice(65 * hh, 65 * (hh + 1))
                o = ps_o.tile([65, FD], f32, tag="o", name=f"o_{p}_{hh}_{qh}")

                def emit_scores_exp(i):
                    ks = slice(128 * i, 128 * (i + 1))
                    s = ps_s.tile([P, FD], f32, tag="s",
                                  name=f"s_{p}_{hh}_{qh}_{i}")
                    for j in range(2):
                        js = slice(512 * j, 512 * (j + 1))
                        qj = slice(FD * qh + 512 * j, FD * qh + 512 * (j + 1))
                        nc.tensor.matmul(
                            s[:, js], lhsT=kT[p][:, hh, ks],
                            rhs=qT[p][:, hh, qj], start=True, stop=True)
                    e = expp.tile([P, FD], f16, tag="exp",
                                  name=f"e_{p}_{hh}_{qh}_{i}")
                    nc.scalar.activation(e[:], s[:], EXP)
                    return e

                def emit_attnv(i, e):
                    for j in range(2):
                        js = slice(512 * j, 512 * (j + 1))
                        nc.tensor.matmul(
                            o[:, js], lhsT=vv[p][:, i, vs], rhs=e[:, js],
                            start=(i == 0), stop=(i == NT - 1))

                # attnv runs two blocks behind scores so a filler that
                # briefly stalls (PSUM buffer not yet free, eviction chain)
                # never delays the next block's scores -> exp
                pending = []
                for i in range(NT):
                    pending.append((i, emit_scores_exp(i)))
                    for f in fillers[i]:
                        f()
                    if len(pending) > 2:
                        bi, e = pending.pop(0)
                        emit_attnv(bi, e)
                for f in fillers[NT]:
                    f()
                for bi, e in pending:
                    emit_attnv(bi, e)

                norm_head(p, hh, qh, o)

            # ---- critical prefix: just what section (0,0,0) block 0 needs
            # plus v0..v2 (v_j is consumed by attnv(j), two blocks behind) ----
            emit_qk_chunk(0, wk_sb, kT[0], 0)
            emit_qk_chunk(0, wq_sb, qT[0], 0)
            emit_qk_chunk(0, wq_sb, qT[0], 1)
            emit_v_tile(0)
            emit_v_tile(1)
            emit_v_tile(2)

            # ---- filler thunks ----
            def f_v(tt):
                return lambda: emit_v_tile(tt)

            def f_k(p, ch):
                return lambda: emit_qk_chunk(0 if p == 0 else 1, wk_sb, kT[p],
                                             ch, dve_only=True)

            def f_q(p, ch):
                return lambda: emit_qk_chunk(0 if p == 0 else 1, wq_sb, qT[p],
                                             ch, dve_only=True)

            def f_y(tt, act_evict=False):
                return lambda: emit_y_block(tt, act_evict)

            def plan(assignments):
                """assignments: {block_index: [thunks]} -> per-block lists"""
                out = [[] for _ in range(NT + 1)]
                for i, ths in assignments.items():
                    out[i] = ths
                return out

            # sections run qh-major: all qh=0 first, then qh=1, so the
            # y-projection (needs every head for a token range) can start
            # halfway through.
            #
            # deps: scores(blk i) needs kT chunk i//4; attnv(i) (emitted in
            # blk i+1 slot) needs v tile i; section idx 2 needs kT[1]+qT[1]
            # chunks 0,1; sections 4+ need qT[*] chunks 2,3; y blocks 0..7
            # need all qh=0 norms (after section 3).
            sec_fillers = {
                # (0,0,0): one PSUM-allocating filler per block. v_j is due
                # at pop-slot j+1 (consumed at block j+2); kT[0] chunk c is
                # due ~2 blocks before block 4c's scores
                0: plan({0: [f_v(3)], 1: [f_v(4)], 2: [f_k(0, 1)],
                         3: [f_v(5)], 4: [f_v(6)], 5: [f_v(7)],
                         6: [f_k(0, 2)], 7: [f_v(8)], 8: [f_v(9)],
                         9: [f_v(10)], 10: [f_k(0, 3)], 11: [f_v(11)],
                         12: [f_v(12)], 13: [f_v(13)], 14: [f_v(14)],
                         15: [f_v(15)]}),
                # (0,1,0): pair-1 k/q chunks for section 2
                1: plan({1: [f_k(1, 0)], 3: [f_k(1, 1)], 5: [f_k(1, 2)],
                         7: [f_k(1, 3)], 9: [f_q(1, 0)], 11: [f_q(1, 1)]}),
                # (1,0,0): qT[0] second half for section 4
                2: plan({2: [f_q(0, 2)], 6: [f_q(0, 3)]}),
                # (1,1,0): qT[1] second half for section 6
                3: plan({2: [f_q(1, 2)], 6: [f_q(1, 3)]}),
                # qh=1 sections: y blocks 0..7 (ready after section 3's norm,
                # which needs ~2 blocks to clear the DVE/GpSimd chain)
                4: plan({4: [f_y(0)], 7: [f_y(1)], 10: [f_y(2)],
                         13: [f_y(3)]}),
                5: plan({2: [f_y(4)], 6: [f_y(5)], 10: [f_y(6)],
                         14: [f_y(7)]}),
                6: plan({}),
                7: plan({}),
            }

            sections = [(0, 0, 0), (0, 1, 0), (1, 0, 0), (1, 1, 0),
                        (0, 0, 1), (0, 1, 1), (1, 0, 1), (1, 1, 1)]
            for idx, (p, hh, qh) in enumerate(sections):
                emit_section(p, hh, qh, sec_fillers[idx])

            # ---- tail: y blocks for the second token half ----
            for tt in range(8, NT):
                emit_y_block(tt, act_evict=(tt % 2 == 0))

    nc.finalize()
    return nc


def _get_nc():
    if "nc" not in _cache:
        _cache["nc"] = _build()
    return _cache["nc"]


def _pack(wt, groups):
    # [G*128, M] row-major -> [128, G*M]: partition p holds the concat over
    # groups of row (g*128 + p), so the DMA reads one contiguous run per p
    g128, m = wt.shape
    assert g128 == groups * 128
    return np.ascontiguousarray(
        wt.reshape(groups, 128, m).transpose(1, 0, 2).reshape(128, groups * m))


def _make_in_maps(x, q_w, kv_w, proj_w):
    x = np.asarray(x, dtype=np.float32)
    q_w = np.asarray(q_w, dtype=np.float32)
    kv_w = np.asarray(kv_w, dtype=np.float32)
    proj_w = np.asarray(proj_w, dtype=np.float32)
    f16 = np.float16
    in_maps = []
    for core in range(NCORES):
        b, g = core // 2, core % 2
        hs = slice(g * 256, (g + 1) * 256)
        in_maps.append({
            "xT": np.ascontiguousarray(x[b].T.astype(f16)),
            "wqT": _pack((q_w[hs, :] * np.float32(SCALE)).T.astype(f16), CT),
            "wkT": _pack(kv_w[hs, :].T.astype(f16), CT),
            "wvT": _pack(
                kv_w[C + g * 256:C + (g + 1) * 256, :].T.astype(f16), CT),
            "pwT": _pack(proj_w[:, hs].T.astype(f16), 2),
        })
    return in_maps


def kernel(x, q_w, kv_w, proj_w, proj_b, H=None, W=None, _trace=False):
    from concourse.bass_utils import run_bass_kernel_spmd

    nc = _get_nc()
    in_maps = _make_in_maps(x, q_w, kv_w, proj_w)
    res = run_bass_kernel_spmd(nc, in_maps, core_ids=list(range(NCORES)),
                               trace=_trace)
    proj_b = np.asarray(proj_b, dtype=np.float32)
    out = np.empty((B, N, C), dtype=np.float32)
    for b in range(B):
        out[b] = res.results[2 * b]["y"] + res.results[2 * b + 1]["y"] + proj_b
    if _trace:
        return out, res
    return out
